# revision 30
# baseline (speedup 1.0000x reference)
"""CIN (xDeepFM compressed interaction network) kernel for Trainium2.

Reference computation (per batch b, embedding dim d):
  h1[b,h,d] = sum_{i,j} x[b,i,d] * x[b,j,d]  * W0[i*39+j, h]   i,j < 39
  h2[b,h,d] = sum_{i,j} x[b,i,d] * h1[b,j,d] * W1[i*128+j, h]  i < 39, j < 128
  h3[b,h,d] = sum_{i,j} x[b,i,d] * h2[b,j,d] * W2[i*128+j, h]
  out[b, :] = concat(sum_d h1, sum_d h2, sum_d h3)   -> [B, 384]

Strategy: data-parallel over batch on 8 cores (512 batches each). On-chip
layout is feature-on-partitions / (b,d)-on-free-dim, fp16 compute with fp32
PSUM accumulation, fully fused across the three layers (h1/h2 never touch
HBM).

Layer 1 exploits symmetry of x (x) x: W0 is folded host-side to the upper
triangle (780 pairs, padded to 117x8), so layer 1 costs 8 matmul passes
instead of 13. Its elementwise operand patterns are host-packed.

Layer 2 builds outer-product "Z" tiles with DVE fp16 multiplies against
x-rows replicated across 128 partitions by a DMA broadcast (chunked in 3
pieces per tile for pipelining); per-batch d-sums of h1/h2 are single DVE
strided reductions straight out of PSUM.

Layer 3 is never materialized: only sum_d h3 is needed, so per-batch Gram
matrices G2[b,j,i] = sum_d h2_j x_i are formed with 4 paired PE transposes
(two batches per [128,128] transpose) + 8 small matmuls against a
host-transposed x tile, and contracted with W2 once at the end.
"""

import sys

sys.path.insert(0, "/opt/trn_rl_repo")

import numpy as np

M = 39          # fields
D = 64          # embedding dim
H = 128         # hidden per CIN layer
B_TOTAL = 4096
N_CORES = 8
B_CORE = B_TOTAL // N_CORES      # 512 batches per core
TILE_B = 8                       # batches per tile
TILE_N = TILE_B * D              # 512 columns per tile
L1_CHUNK = 117                   # partition rows per layer-1 chunk
L1_K = 8                         # layer-1 i-slots per row (j fixed per row)
BC_CHUNK = 13                    # i-rows per broadcast chunk (3 chunks = 39)

_NC_CACHE = {}

# upper-triangle pair enumeration for layer 1, row-major into [117, 8]
# layer-1 row assignment: each of 117 partition-rows has a FIXED j and up
# to L1_K i-values (i <= j).  780 upper-triangle pairs -> 115 rows.
def _l1_rows():
    rows = []  # (j, [i...])
    for j in range(M):
        for i0 in range(0, j + 1, L1_K):
            rows.append((j, list(range(i0, min(i0 + L1_K, j + 1)))))
    assert len(rows) <= L1_CHUNK, len(rows)
    rows += [(0, [])] * (L1_CHUNK - len(rows))
    return rows

_ROWS = _l1_rows()


def _build(b_core):
    import concourse.bacc as bacc
    import concourse.tile as tile
    from concourse import mybir
    from concourse.masks import make_identity

    f32 = mybir.dt.float32
    f16 = mybir.dt.float16

    n_tiles = b_core // TILE_B

    nc = bacc.Bacc("TRN2", target_bir_lowering=False, debug=False)
    # host-prepared tensors (fp16, pre-arranged); see kernel() below
    xt16_d = nc.dram_tensor(
        "xt16", [n_tiles, M, TILE_N], f16, kind="ExternalInput"
    )
    # per-slot permutation matrices: pmat[i, k, r] = 1 iff ii[r, k] == i
    # for k < L1_K; slot L1_K is the j-gather (xjp) permutation
    pmat_d = nc.dram_tensor(
        "pmat", [M, L1_K + 1, L1_CHUNK], f16, kind="ExternalInput"
    )
    # x transposed per batch-pair: partition = (b%2)*64 + d, chunk = b//2,
    # zero-padded on the wrong half so full-K matmuls select one batch
    xdt_d = nc.dram_tensor(
        "xdt", [n_tiles, H, TILE_B // 2, 2, M], f16, kind="ExternalInput"
    )
    w0_d = nc.dram_tensor(
        "W0s", [L1_CHUNK, L1_K, H], f16, kind="ExternalInput"
    )
    w1_d = nc.dram_tensor("W1t", [H, M, H], f16, kind="ExternalInput")
    w2_d = nc.dram_tensor("W2t", [H, M, H], f16, kind="ExternalInput")
    out_d = nc.dram_tensor("out", [3, H, b_core], f32, kind="ExternalOutput")

    with tile.TileContext(nc) as tc:
        with tc.tile_pool(name="resident", bufs=1) as resident:
            w0_sb = resident.tile([L1_CHUNK, L1_K, H], f16)
            nc.sync.dma_start(w0_sb[:], w0_d.ap())
            w1_sb = resident.tile([H, M, H], f16)
            nc.sync.dma_start(w1_sb[:], w1_d.ap())
            w2_sb = resident.tile([H, M, H], f16)
            nc.sync.dma_start(w2_sb[:], w2_d.ap())
            identity = resident.tile([H, H], f16)
            make_identity(nc, identity[:])
            pmat_sb = resident.tile([M, L1_K + 1, L1_CHUNK], f16)
            nc.sync.dma_start(pmat_sb[:], pmat_d.ap())

            # per-core accumulated outputs
            out_sb = resident.tile([H, 2, b_core], f32)
            out3_sb = resident.tile([H, b_core], f32)
            # G2 gram results, [j, b, i]
            g2t_sb = resident.tile([H, b_core, M], f16)

            xt16_ap = xt16_d.ap()  # [n_tiles, M, TILE_N]
            with (
                tc.tile_pool(name="pat", bufs=5) as pat,
                tc.tile_pool(name="patip", bufs=2) as patip,
                tc.tile_pool(name="z1pool", bufs=2) as z1pool,
                tc.tile_pool(name="z2pool", bufs=2) as z2pool,
                tc.tile_pool(name="hsb", bufs=2) as hsb,
                tc.tile_pool(name="gram", bufs=2) as gram,
                tc.tile_pool(name="psum", bufs=2, space="PSUM") as psum,
                tc.tile_pool(name="psum_g", bufs=1, space="PSUM") as psum_g,
                tc.tile_pool(name="psum_x", bufs=1, space="PSUM") as psum_x,
            ):
                # layer 3 via per-batch Gram matrices, deferred by one tile
                # so its PE work fills the bubble while the next tile's Z
                # tiles are being built on DVE
                def gram_phase(t, xdt_t, h2_16):
                    # paired transposes: chunk c covers batches 2c, 2c+1
                    # h2t[p, c, j]: p = (b%2)*64 + d
                    h2t_ps = psum_g.tile([H, 4, H], f16, tag="h2tps")
                    for c in range(4):
                        nc.tensor.transpose(
                            h2t_ps[:, c, :],
                            h2_16[:, c * H : (c + 1) * H],
                            identity[:],
                        )
                    h2dt = gram.tile([H, 4, H], f16, tag="h2dt")
                    nc.scalar.activation(
                        h2dt[:], h2t_ps[:],
                        mybir.ActivationFunctionType.Copy,
                    )
                    # G2[j, i | b] = sum_d h2[j, d] * x[i, d]; one matmul per
                    # batch-pair (xdt zero-padding selects the right batch)
                    g2_ps = psum_g.tile([H, TILE_B, M], f32, tag="g2ps")
                    for c in range(TILE_B // 2):
                        nc.tensor.matmul(
                            g2_ps[:, 2 * c : 2 * c + 2, :],
                            h2dt[:, c, :],
                            xdt_t[:, c, :, :],
                            start=True, stop=True,
                        )
                    nc.scalar.activation(
                        g2t_sb[:, t * TILE_B : (t + 1) * TILE_B, :],
                        g2_ps[:],
                        mybir.ActivationFunctionType.Copy,
                    )

                prev_gram = None
                for t in range(n_tiles):
                    # x rows replicated across partitions, 3 chunks of 13:
                    # bc[c][p, i, :] = x^T[13c+i, tile t] for all p
                    bcs = []
                    for c in range(3):
                        bc = pat.tile([H, BC_CHUNK, TILE_N], f16, tag="bc")
                        eng = nc.sync if c % 2 == 0 else nc.scalar
                        eng.dma_start(
                            bc[:],
                            xt16_ap[t, c * BC_CHUNK : (c + 1) * BC_CHUNK]
                            .rearrange("i c -> (i c)")[None]
                            .to_broadcast([H, BC_CHUNK * TILE_N]),
                        )
                        bcs.append(bc)
                    # layer-1 operand patterns generated on-chip by
                    # permutation matmuls (saves 69 MB/core of HBM traffic)
                    xt_t = patip.tile([M, TILE_N], f16, tag="xt")
                    nc.scalar.dma_start(xt_t[:], xt16_ap[t])
                    x_ip = patip.tile([L1_CHUNK, L1_K, TILE_N], f16, tag="ip")
                    x_jp = patip.tile([L1_CHUNK, TILE_N], f16, tag="jp")
                    for q in range(4):
                        xp_ps = psum_x.tile([L1_CHUNK, 2, TILE_N], f32, tag="xp")
                        for u in range(2):
                            k = 2 * q + u
                            nc.tensor.matmul(
                                xp_ps[:, u, :],
                                pmat_sb[:, k, :],
                                xt_t[:],
                                start=True, stop=True,
                            )
                        nc.scalar.activation(
                            x_ip[:, 2 * q : 2 * q + 2, :],
                            xp_ps[:],
                            mybir.ActivationFunctionType.Copy,
                        )
                    xp_ps = psum_x.tile([L1_CHUNK, 2, TILE_N], f32, tag="xp")
                    nc.tensor.matmul(
                        xp_ps[:, 0, :],
                        pmat_sb[:, L1_K, :],
                        xt_t[:],
                        start=True, stop=True,
                    )
                    nc.scalar.activation(
                        x_jp[:],
                        xp_ps[:, 0, :],
                        mybir.ActivationFunctionType.Copy,
                    )
                    # host-transposed x for the layer-3 gram
                    xdt_t = gram.tile([H, TILE_B // 2, 2, M], f16, tag="xdt")
                    nc.sync.dma_start(xdt_t[:], xdt_d.ap()[t])

                    # deferred layer-3 of the previous tile
                    if prev_gram is not None:
                        gram_phase(*prev_gram)
                    prev_gram = None

                    # ---- layer 1 (symmetrized) ----
                    h1_ps = psum.tile([H, TILE_N], f32, tag="h1ps")
                    for k0 in range(0, L1_K, 4):
                        z1 = z1pool.tile(
                            [L1_CHUNK, 4, TILE_N], f16, tag="z1"
                        )
                        nc.vector.tensor_mul(
                            z1[:],
                            x_ip[:, k0 : k0 + 4, :],
                            x_jp[:, None, :].broadcast_to(
                                [L1_CHUNK, 4, TILE_N]
                            ),
                        )
                        for u in range(4):
                            k = k0 + u
                            nc.tensor.matmul(
                                h1_ps[:],
                                w0_sb[:, k, :],
                                z1[:, u, :],
                                start=(k == 0),
                                stop=(k == L1_K - 1),
                            )
                    h1_16 = hsb.tile([H, TILE_N], f16, tag="h1")
                    nc.scalar.activation(
                        h1_16[:], h1_ps[:],
                        mybir.ActivationFunctionType.Copy,
                    )
                    nc.vector.reduce_sum(
                        out_sb[:, 0, t * TILE_B : (t + 1) * TILE_B],
                        h1_ps[:].rearrange("p (b d) -> p b d", b=TILE_B),
                        axis=mybir.AxisListType.X,
                    )

                    # ---- layer 2 ----
                    h2_ps = psum.tile([H, TILE_N], f32, tag="h2ps")
                    for c in range(3):
                        z2 = z2pool.tile(
                            [H, BC_CHUNK, TILE_N], f16, tag="z2"
                        )
                        nc.vector.tensor_mul(
                            z2[:],
                            bcs[c][:],
                            h1_16[:, None, :].broadcast_to(
                                [H, BC_CHUNK, TILE_N]
                            ),
                        )
                        for u in range(BC_CHUNK):
                            i = c * BC_CHUNK + u
                            nc.tensor.matmul(
                                h2_ps[:],
                                w1_sb[:, i, :],
                                z2[:, u, :],
                                start=(i == 0),
                                stop=(i == M - 1),
                            )
                    h2_16 = hsb.tile([H, TILE_N], f16, tag="h2")
                    nc.scalar.activation(
                        h2_16[:], h2_ps[:],
                        mybir.ActivationFunctionType.Copy,
                    )
                    nc.vector.reduce_sum(
                        out_sb[:, 1, t * TILE_B : (t + 1) * TILE_B],
                        h2_ps[:].rearrange("p (b d) -> p b d", b=TILE_B),
                        axis=mybir.AxisListType.X,
                    )

                    prev_gram = (t, xdt_t, h2_16)

                    # first-half final contraction + output store, issued
                    # mid-loop so they overlap remaining tiles' DMA/compute
                    if t == n_tiles // 2 + 1:
                        half = b_core // 2
                        o3a_ps = psum.tile([H, half], f32, tag="h2ps")
                        for i in range(M):
                            nc.tensor.matmul(
                                o3a_ps[:],
                                w2_sb[:, i, :],
                                g2t_sb[:, :half, i],
                                start=(i == 0),
                                stop=(i == M - 1),
                            )
                        nc.vector.tensor_copy(out3_sb[:, :half], o3a_ps[:])
                        nc.sync.dma_start(
                            out_d.ap()[0:2, :, :half].rearrange(
                                "l h b -> h l b"
                            ),
                            out_sb[:, :, :half],
                        )
                        nc.sync.dma_start(
                            out_d.ap()[2, :, :half], out3_sb[:, :half]
                        )

                gram_phase(*prev_gram)

                # ---- second-half final contraction: out3 = W2^T @ G2T ----
                half = b_core // 2
                out3_ps = psum.tile([H, half], f32, tag="h1ps")
                for i in range(M):
                    nc.tensor.matmul(
                        out3_ps[:],
                        w2_sb[:, i, :],
                        g2t_sb[:, half:, i],
                        start=(i == 0),
                        stop=(i == M - 1),
                    )
                nc.vector.tensor_copy(out3_sb[:, half:], out3_ps[:])

            half = b_core // 2
            nc.sync.dma_start(
                out_d.ap()[0:2, :, half:].rearrange("l h b -> h l b"),
                out_sb[:, :, half:],
            )
            nc.sync.dma_start(out_d.ap()[2, :, half:], out3_sb[:, half:])
    nc.compile()
    return nc


def _get_nc(b_core):
    if b_core not in _NC_CACHE:
        _NC_CACHE[b_core] = _build(b_core)
    return _NC_CACHE[b_core]


_IDX = None


def _pair_index():
    """Per-row j, and the on-chip xip permutation matrices
    pmat[i, k, r] = 1 iff row r slot k gathers x_i (pad slots all-zero)."""
    global _IDX
    if _IDX is None:
        pmat = np.zeros((M, L1_K + 1, L1_CHUNK), np.float16)
        for r, (j, ilist) in enumerate(_ROWS):
            for k, i in enumerate(ilist):
                pmat[i, k, r] = 1.0
            if ilist:
                pmat[j, L1_K, r] = 1.0
        _IDX = pmat
    return _IDX


def _pack_weights(W0, W1, W2):
    w0r = W0.reshape(M, M, H).astype(np.float32)
    w0s = np.zeros((L1_CHUNK, L1_K, H), np.float32)
    for r, (j, ilist) in enumerate(_ROWS):
        for k, i in enumerate(ilist):
            w0s[r, k] = w0r[i, j] + (w0r[j, i] if i != j else 0.0)
    w0s = w0s.astype(np.float16)
    w1t = np.ascontiguousarray(
        W1.reshape(M, H, H).transpose(1, 0, 2)
    ).astype(np.float16)
    w2t = np.ascontiguousarray(
        W2.reshape(M, H, H).transpose(1, 0, 2)
    ).astype(np.float16)
    return w0s, w1t, w2t


def kernel(x, W0, W1, W2, _trace=False):
    from concourse.bass_utils import run_bass_kernel_spmd

    x = np.ascontiguousarray(x, dtype=np.float32)
    w0s, w1t, w2t = _pack_weights(W0, W1, W2)

    nc = _get_nc(B_CORE)
    n_tiles = B_CORE // TILE_B
    bd = B_CORE * D
    pmat = _pair_index()
    in_maps = []
    for c in range(N_CORES):
        xc = x[c * B_CORE : (c + 1) * B_CORE]
        xtr = xc.transpose(1, 0, 2).reshape(M, bd).astype(np.float16)
        xt16t = np.ascontiguousarray(
            xtr.reshape(M, n_tiles, TILE_N).transpose(1, 0, 2)
        )  # [n_tiles, M, TILE_N]
        # xdt[t, q*64+d, c, q, i] = x[t*8+2c+q, i, d]; other half zero
        x5 = xc.reshape(n_tiles, TILE_B // 2, 2, M, D).astype(np.float16)
        xdt = np.zeros((n_tiles, 2, D, TILE_B // 2, 2, M), np.float16)
        for q in (0, 1):
            xdt[:, q, :, :, q, :] = x5[:, :, q, :, :].transpose(0, 3, 1, 2)
        xdt = np.ascontiguousarray(
            xdt.reshape(n_tiles, H, TILE_B // 2, 2, M)
        )
        in_maps.append(
            {
                "xt16": xt16t,
                "xdt": xdt,
                "pmat": pmat,
                "W0s": w0s,
                "W1t": w1t,
                "W2t": w2t,
            }
        )
    res = run_bass_kernel_spmd(
        nc, in_maps, core_ids=list(range(N_CORES)), trace=_trace
    )
    # per-core out: [3, H, B_CORE] -> [B_CORE, 3*H]
    outs = []
    for c in range(N_CORES):
        o = res.results[c]["out"]
        outs.append(o.reshape(3 * H, B_CORE).T.reshape(B_CORE, 3 * H))
    full = np.concatenate(outs, axis=0).astype(np.float32)
    if _trace:
        return full, res
    return full


# revision 37
# speedup vs baseline: 1.0698x; 1.0698x over previous
"""CIN (xDeepFM compressed interaction network) kernel for Trainium2.

Reference computation (per batch b, embedding dim d):
  h1[b,h,d] = sum_{i,j} x[b,i,d] * x[b,j,d]  * W0[i*39+j, h]   i,j < 39
  h2[b,h,d] = sum_{i,j} x[b,i,d] * h1[b,j,d] * W1[i*128+j, h]  i < 39, j < 128
  h3[b,h,d] = sum_{i,j} x[b,i,d] * h2[b,j,d] * W2[i*128+j, h]
  out[b, :] = concat(sum_d h1, sum_d h2, sum_d h3)   -> [B, 384]

Strategy: data-parallel over batch on 8 cores (512 batches each). On-chip
layout is feature-on-partitions / (b,d)-on-free-dim, fp16 compute with fp32
PSUM accumulation, fully fused across the three layers (h1/h2 never touch
HBM).

Layer 1 exploits symmetry of x (x) x: W0 is folded host-side to the upper
triangle (780 pairs, padded to 117x8), so layer 1 costs 8 matmul passes
instead of 13. Its elementwise operand patterns are host-packed.

Layer 2 builds outer-product "Z" tiles with DVE fp16 multiplies against
x-rows replicated across 128 partitions by a DMA broadcast (chunked in 3
pieces per tile for pipelining); per-batch d-sums of h1/h2 are single DVE
strided reductions straight out of PSUM.

Layer 3 is never materialized: only sum_d h3 is needed, so per-batch Gram
matrices G2[b,j,i] = sum_d h2_j x_i are formed with 4 paired PE transposes
(two batches per [128,128] transpose) + 8 small matmuls against a
host-transposed x tile, and contracted with W2 once at the end.
"""

import sys

sys.path.insert(0, "/opt/trn_rl_repo")

import numpy as np

M = 39          # fields
D = 64          # embedding dim
H = 128         # hidden per CIN layer
B_TOTAL = 4096
N_CORES = 8
B_CORE = B_TOTAL // N_CORES      # 512 batches per core
TILE_B = 8                       # batches per tile
TILE_N = TILE_B * D              # 512 columns per tile
L1_CHUNK = 117                   # partition rows per layer-1 chunk
L1_K = 8                         # layer-1 i-slots per row (j fixed per row)
BC_CHUNK = 13                    # i-rows per broadcast chunk (3 chunks = 39)

_NC_CACHE = {}

# upper-triangle pair enumeration for layer 1, row-major into [117, 8]
# layer-1 row assignment: each of 117 partition-rows has a FIXED j and up
# to L1_K i-values (i <= j).  780 upper-triangle pairs -> 115 rows.
def _l1_rows():
    rows = []  # (j, [i...])
    for j in range(M):
        for i0 in range(0, j + 1, L1_K):
            rows.append((j, list(range(i0, min(i0 + L1_K, j + 1)))))
    assert len(rows) <= L1_CHUNK, len(rows)
    rows += [(0, [])] * (L1_CHUNK - len(rows))
    return rows

_ROWS = _l1_rows()


def _build(b_core):
    import concourse.bacc as bacc
    import concourse.tile as tile
    from concourse import mybir
    from concourse.masks import make_identity

    f32 = mybir.dt.float32
    f16 = mybir.dt.float16

    n_tiles = b_core // TILE_B

    nc = bacc.Bacc("TRN2", target_bir_lowering=False, debug=False)
    # host-prepared tensors (fp16, pre-arranged); see kernel() below
    xt16_d = nc.dram_tensor(
        "xt16", [n_tiles, M, TILE_N], f16, kind="ExternalInput"
    )
    xjp_d = nc.dram_tensor(
        "xjp", [n_tiles, L1_CHUNK, TILE_N], f16, kind="ExternalInput"
    )
    # per-slot permutation matrices: pmat[i, k, r] = 1 iff ii[r, k] == i
    pmat_d = nc.dram_tensor(
        "pmat", [M, L1_K, L1_CHUNK], f16, kind="ExternalInput"
    )
    # x transposed per batch-pair: partition = (b%2)*64 + d, chunk = b//2,
    # zero-padded on the wrong half so full-K matmuls select one batch
    xdt_d = nc.dram_tensor(
        "xdt", [n_tiles, H, TILE_B // 2, 2, M], f16, kind="ExternalInput"
    )
    w0_d = nc.dram_tensor(
        "W0s", [L1_CHUNK, L1_K, H], f16, kind="ExternalInput"
    )
    w1_d = nc.dram_tensor("W1t", [H, M, H], f16, kind="ExternalInput")
    w2_d = nc.dram_tensor("W2t", [H, M, H], f16, kind="ExternalInput")
    out_d = nc.dram_tensor("out", [3, H, b_core], f32, kind="ExternalOutput")

    with tile.TileContext(nc) as tc:
        with tc.tile_pool(name="resident", bufs=1) as resident:
            w0_sb = resident.tile([L1_CHUNK, L1_K, H], f16)
            nc.sync.dma_start(w0_sb[:], w0_d.ap())
            w1_sb = resident.tile([H, M, H], f16)
            nc.sync.dma_start(w1_sb[:], w1_d.ap())
            w2_sb = resident.tile([H, M, H], f16)
            nc.sync.dma_start(w2_sb[:], w2_d.ap())
            identity = resident.tile([H, H], f16)
            make_identity(nc, identity[:])
            pmat_sb = resident.tile([M, L1_K, L1_CHUNK], f16)
            nc.sync.dma_start(pmat_sb[:], pmat_d.ap())

            # per-core accumulated outputs
            out_sb = resident.tile([H, 2, b_core], f32)
            out3_sb = resident.tile([H, b_core], f32)
            # G2 gram results, [j, b, i]
            g2t_sb = resident.tile([H, b_core, M], f16)

            xt16_ap = xt16_d.ap()  # [n_tiles, M, TILE_N]
            with (
                tc.tile_pool(name="pat", bufs=5) as pat,
                tc.tile_pool(name="patip", bufs=2) as patip,
                tc.tile_pool(name="z1pool", bufs=2) as z1pool,
                tc.tile_pool(name="z2pool", bufs=2) as z2pool,
                tc.tile_pool(name="hsb", bufs=2) as hsb,
                tc.tile_pool(name="gram", bufs=2) as gram,
                tc.tile_pool(name="psum", bufs=2, space="PSUM") as psum,
                tc.tile_pool(name="psum_g", bufs=1, space="PSUM") as psum_g,
                tc.tile_pool(name="psum_x", bufs=1, space="PSUM") as psum_x,
            ):
                # layer 3 via per-batch Gram matrices, deferred by one tile
                # so its PE work fills the bubble while the next tile's Z
                # tiles are being built on DVE
                def gram_phase(t, xdt_t, h2_16):
                    # paired transposes: chunk c covers batches 2c, 2c+1
                    # h2t[p, c, j]: p = (b%2)*64 + d
                    h2t_ps = psum_g.tile([H, 4, H], f16, tag="h2tps")
                    for c in range(4):
                        nc.tensor.transpose(
                            h2t_ps[:, c, :],
                            h2_16[:, c * H : (c + 1) * H],
                            identity[:],
                        )
                    h2dt = gram.tile([H, 4, H], f16, tag="h2dt")
                    nc.scalar.activation(
                        h2dt[:], h2t_ps[:],
                        mybir.ActivationFunctionType.Copy,
                    )
                    # G2[j, i | b] = sum_d h2[j, d] * x[i, d]; one matmul per
                    # batch-pair (xdt zero-padding selects the right batch)
                    g2_ps = psum_g.tile([H, TILE_B, M], f32, tag="g2ps")
                    for c in range(TILE_B // 2):
                        nc.tensor.matmul(
                            g2_ps[:, 2 * c : 2 * c + 2, :],
                            h2dt[:, c, :],
                            xdt_t[:, c, :, :],
                            start=True, stop=True,
                        )
                    nc.scalar.activation(
                        g2t_sb[:, t * TILE_B : (t + 1) * TILE_B, :],
                        g2_ps[:],
                        mybir.ActivationFunctionType.Copy,
                    )

                prev_gram = None
                for t in range(n_tiles):
                    # x rows replicated across partitions, 3 chunks of 13:
                    # bc[c][p, i, :] = x^T[13c+i, tile t] for all p
                    bcs = []
                    for c in range(3):
                        bc = pat.tile([H, BC_CHUNK, TILE_N], f16, tag="bc")
                        eng = nc.sync if c % 2 == 0 else nc.scalar
                        eng.dma_start(
                            bc[:],
                            xt16_ap[t, c * BC_CHUNK : (c + 1) * BC_CHUNK]
                            .rearrange("i c -> (i c)")[None]
                            .to_broadcast([H, BC_CHUNK * TILE_N]),
                        )
                        bcs.append(bc)
                    # layer-1 operand patterns: xjp from HBM; xip generated
                    # on-chip by permutation matmuls (saves 61 MB/core HBM)
                    x_jp = patip.tile([L1_CHUNK, TILE_N], f16, tag="jp")
                    nc.sync.dma_start(x_jp[:], xjp_d.ap()[t])
                    xt_t = patip.tile([M, TILE_N], f16, tag="xt")
                    nc.scalar.dma_start(xt_t[:], xt16_ap[t])
                    x_ip = patip.tile([L1_CHUNK, L1_K, TILE_N], f16, tag="ip")
                    for q in range(4):
                        xp_ps = psum_x.tile([L1_CHUNK, 2, TILE_N], f32, tag="xp")
                        for u in range(2):
                            k = 2 * q + u
                            nc.tensor.matmul(
                                xp_ps[:, u, :],
                                pmat_sb[:, k, :],
                                xt_t[:],
                                start=True, stop=True,
                            )
                        nc.scalar.activation(
                            x_ip[:, 2 * q : 2 * q + 2, :],
                            xp_ps[:],
                            mybir.ActivationFunctionType.Copy,
                        )
                    # host-transposed x for the layer-3 gram
                    xdt_t = gram.tile([H, TILE_B // 2, 2, M], f16, tag="xdt")
                    nc.sync.dma_start(xdt_t[:], xdt_d.ap()[t])

                    # deferred layer-3 of the previous tile
                    if prev_gram is not None:
                        gram_phase(*prev_gram)
                    prev_gram = None

                    # ---- layer 1 (symmetrized) ----
                    h1_ps = psum.tile([H, TILE_N], f32, tag="h1ps")
                    for k0 in range(0, L1_K, 4):
                        z1 = z1pool.tile(
                            [L1_CHUNK, 4, TILE_N], f16, tag="z1"
                        )
                        nc.vector.tensor_mul(
                            z1[:],
                            x_ip[:, k0 : k0 + 4, :],
                            x_jp[:, None, :].broadcast_to(
                                [L1_CHUNK, 4, TILE_N]
                            ),
                        )
                        for u in range(4):
                            k = k0 + u
                            nc.tensor.matmul(
                                h1_ps[:],
                                w0_sb[:, k, :],
                                z1[:, u, :],
                                start=(k == 0),
                                stop=(k == L1_K - 1),
                            )
                    h1_16 = hsb.tile([H, TILE_N], f16, tag="h1")
                    nc.scalar.activation(
                        h1_16[:], h1_ps[:],
                        mybir.ActivationFunctionType.Copy,
                    )
                    nc.vector.reduce_sum(
                        out_sb[:, 0, t * TILE_B : (t + 1) * TILE_B],
                        h1_ps[:].rearrange("p (b d) -> p b d", b=TILE_B),
                        axis=mybir.AxisListType.X,
                    )

                    # ---- layer 2 ----
                    h2_ps = psum.tile([H, TILE_N], f32, tag="h2ps")
                    for c in range(3):
                        z2 = z2pool.tile(
                            [H, BC_CHUNK, TILE_N], f16, tag="z2"
                        )
                        nc.vector.tensor_mul(
                            z2[:],
                            bcs[c][:],
                            h1_16[:, None, :].broadcast_to(
                                [H, BC_CHUNK, TILE_N]
                            ),
                        )
                        for u in range(BC_CHUNK):
                            i = c * BC_CHUNK + u
                            nc.tensor.matmul(
                                h2_ps[:],
                                w1_sb[:, i, :],
                                z2[:, u, :],
                                start=(i == 0),
                                stop=(i == M - 1),
                            )
                    h2_16 = hsb.tile([H, TILE_N], f16, tag="h2")
                    nc.scalar.activation(
                        h2_16[:], h2_ps[:],
                        mybir.ActivationFunctionType.Copy,
                    )
                    nc.vector.reduce_sum(
                        out_sb[:, 1, t * TILE_B : (t + 1) * TILE_B],
                        h2_ps[:].rearrange("p (b d) -> p b d", b=TILE_B),
                        axis=mybir.AxisListType.X,
                    )

                    prev_gram = (t, xdt_t, h2_16)

                    # batches 0..255 are final after tile 31: store them
                    # early so the output DMA overlaps remaining tiles
                    if t == n_tiles // 2 + 1:
                        half = b_core // 2
                        nc.sync.dma_start(
                            out_d.ap()[0:2, :, :half].rearrange(
                                "l h b -> h l b"
                            ),
                            out_sb[:, :, :half],
                        )

                gram_phase(*prev_gram)

                # ---- final contraction: out3 = W2^T @ G2T ----
                # (reuses an h1ps PSUM slot; same shape, loop is done)
                out3_ps = psum.tile([H, b_core], f32, tag="h1ps")
                for i in range(M):
                    nc.tensor.matmul(
                        out3_ps[:],
                        w2_sb[:, i, :],
                        g2t_sb[:, :, i],
                        start=(i == 0),
                        stop=(i == M - 1),
                    )
                nc.vector.tensor_copy(out3_sb[:], out3_ps[:])

            half = b_core // 2
            nc.sync.dma_start(
                out_d.ap()[0:2, :, half:].rearrange("l h b -> h l b"),
                out_sb[:, :, half:],
            )
            nc.sync.dma_start(out_d.ap()[2], out3_sb[:])
    nc.compile()
    return nc


def _get_nc(b_core):
    if b_core not in _NC_CACHE:
        _NC_CACHE[b_core] = _build(b_core)
    return _NC_CACHE[b_core]


_IDX = None


def _pair_index():
    """Per-row j, and the on-chip xip permutation matrices
    pmat[i, k, r] = 1 iff row r slot k gathers x_i (pad slots all-zero)."""
    global _IDX
    if _IDX is None:
        jj = np.array([j for j, _ in _ROWS], np.int64)
        pmat = np.zeros((M, L1_K, L1_CHUNK), np.float16)
        for r, (j, ilist) in enumerate(_ROWS):
            for k, i in enumerate(ilist):
                pmat[i, k, r] = 1.0
        _IDX = (pmat, jj)
    return _IDX


def _pack_weights(W0, W1, W2):
    w0r = W0.reshape(M, M, H).astype(np.float32)
    w0s = np.zeros((L1_CHUNK, L1_K, H), np.float32)
    for r, (j, ilist) in enumerate(_ROWS):
        for k, i in enumerate(ilist):
            w0s[r, k] = w0r[i, j] + (w0r[j, i] if i != j else 0.0)
    w0s = w0s.astype(np.float16)
    w1t = np.ascontiguousarray(
        W1.reshape(M, H, H).transpose(1, 0, 2)
    ).astype(np.float16)
    w2t = np.ascontiguousarray(
        W2.reshape(M, H, H).transpose(1, 0, 2)
    ).astype(np.float16)
    return w0s, w1t, w2t


def kernel(x, W0, W1, W2, _trace=False):
    from concourse.bass_utils import run_bass_kernel_spmd

    x = np.ascontiguousarray(x, dtype=np.float32)
    w0s, w1t, w2t = _pack_weights(W0, W1, W2)

    nc = _get_nc(B_CORE)
    n_tiles = B_CORE // TILE_B
    bd = B_CORE * D
    pmat, jj = _pair_index()
    in_maps = []
    for c in range(N_CORES):
        xc = x[c * B_CORE : (c + 1) * B_CORE]
        xtr = xc.transpose(1, 0, 2).reshape(M, bd).astype(np.float16)
        xt16t = np.ascontiguousarray(
            xtr.reshape(M, n_tiles, TILE_N).transpose(1, 0, 2)
        )  # [n_tiles, M, TILE_N]
        xjp = np.ascontiguousarray(xt16t[:, jj, :])
        # xdt[t, q*64+d, c, q, i] = x[t*8+2c+q, i, d]; other half zero
        x5 = xc.reshape(n_tiles, TILE_B // 2, 2, M, D).astype(np.float16)
        xdt = np.zeros((n_tiles, 2, D, TILE_B // 2, 2, M), np.float16)
        for q in (0, 1):
            xdt[:, q, :, :, q, :] = x5[:, :, q, :, :].transpose(0, 3, 1, 2)
        xdt = np.ascontiguousarray(
            xdt.reshape(n_tiles, H, TILE_B // 2, 2, M)
        )
        in_maps.append(
            {
                "xt16": xt16t,
                "xjp": xjp,
                "xdt": xdt,
                "pmat": pmat,
                "W0s": w0s,
                "W1t": w1t,
                "W2t": w2t,
            }
        )
    res = run_bass_kernel_spmd(
        nc, in_maps, core_ids=list(range(N_CORES)), trace=_trace
    )
    # per-core out: [3, H, B_CORE] -> [B_CORE, 3*H]
    outs = []
    for c in range(N_CORES):
        o = res.results[c]["out"]
        outs.append(o.reshape(3 * H, B_CORE).T.reshape(B_CORE, 3 * H))
    full = np.concatenate(outs, axis=0).astype(np.float32)
    if _trace:
        return full, res
    return full
